# revision 7
# baseline (speedup 1.0000x reference)
"""Distributed Trainium2 kernel for GQA causal attention (B=2, L=2048, DIM=2048,
H=32 q-heads, KV=8 kv-heads, HD=64), tensor-parallel over heads across 8 cores.

v2 layout: everything stays transposed ([dims, seq]) end-to-end.

Per-core pipeline (core r owns q heads 4r..4r+3 and kv head r):
  P1: qkvT = wcomb^T-stationary matmuls streaming x^T (N=512) -> q/k emerge
      directly in [head_dim, seq] layout (no PE transposes); RoPE via 4 DVE
      ops (2 mults against packed cos/sin tables, stream_shuffle pair-swap,
      add); v recovered to [seq, hd] by 32 small PE transposes.
  P2: scores computed transposed (S^T[j,i]) with the two heads of a pair
      packed into the two 64-row halves of the PE array (GQA: they share the
      kv head; stationary is the kv block duplicated to both halves) -> one
      512-cycle stream yields two heads' scores. exp split ACT/DVE
      (Schraudolph+polish custom op), denominator via ones-column of V.
      jt-outer / chunk-inner loop amortizes weight loads.
  P2.5: per-batch AllToAll moves o^T blocks [dims, seq-shard] plus raw
      denominator rows; output lands as [all 2048 dims, my 256 seq] directly
      usable as projection lhsT (no transposes anywhere).
  P3: normalize lhs tiles with a K=2 broadcast matmul of 1/den, then full
      output projection for this core's seq shard.
Host: concatenates the 8 row-shards.
"""

import sys

sys.path.insert(0, "/opt/trn_rl_repo")

import numpy as np
import ml_dtypes

from concourse import bass, bacc, mybir, tile
from concourse.bass_utils import run_bass_kernel_spmd

# ---- custom DVE op: Schraudolph exp + parabola mantissa polish ------------
# out = Src1 * ((f^2 - |f|) + 1/k), f = Src0 - rint(Src0) via the magic-add;
# since f^2-|f| = fhat^2-fhat for fhat = frac(Src0), this multiplies the
# Schraudolph value y = 2^x*(1+fhat)/2^fhat by (1 + k*(fhat^2-fhat))/k, a
# two-parameter fit of 2^fhat/(1+fhat) (0.4% rms). The global 1/k scale
# cancels in the softmax ratio; ScalarE pieces match it via an exp bias.
from concourse import dve_ops as _D
from concourse.dve_spec import (AluOp as _AluOp, Bin as _Bin, Spec as _Spec,
                                Src0 as _Src0, Src1 as _Src1, C0 as _C0,
                                C1 as _C1, Zero as _Zero, sq as _sq,
                                lower as _dve_lower)
from concourse.dve_uop import DveOpSpec as _DveOpSpec

MAGIC = 12582912.0
KPAR = 0.2150
INVK = 1.0 / KPAR
ASCALE = 0.996908
EXP_A1 = float(np.float32(2 ** 23))
EXP_B1 = float(np.float32(127 * 2 ** 23) + np.float32(2 ** 23 * np.log2(ASCALE)))
LOG2E = float(np.float64(1.0) / np.log(2.0))
ACT_EXP_SCALE = float(np.log(2.0))
ACT_EXP_BIAS = float(np.log(INVK))


def _polish_ref(in0, in1, s0, s1, imm2):
    f = (in0 - np.rint(in0)).astype(np.float32)
    return ((f * f - np.abs(f)) + np.float32(s1)) * in1


def _install_polish():
    for op in _D.OPS:
        if op.name == "EXP_POLISH_ANT":
            return op
    f = _Bin(_AluOp.SUBTRACT, _Src0,
             _Bin(_AluOp.SUBTRACT, _Bin(_AluOp.ADD, _Src0, _C0), _C0))
    g = _Bin(_AluOp.SUBTRACT, _sq(f), _Bin(_AluOp.ABSOLUTE_DIFF, f, _Zero))
    body = _Bin(_AluOp.MULTIPLY, _Bin(_AluOp.ADD, g, _C1), _Src1)
    row = _D._CUSTOM_DVE_ROW_BASE + len(_D.OPS)
    spec = _Spec(body=body, reference=_polish_ref)
    sha = _DveOpSpec(name="EXP_POLISH_ANT", opcode=row,
                     uops=_dve_lower(spec, ver="v3"), rd1_en=True).sha("v3")
    op = _D.DveOp("EXP_POLISH_ANT", spec, subdim=False,
                  uops_sha={"v3": sha, "v4": sha})
    _D.OPS.append(op)
    _D._SUB_OPCODE_FOR_NAME["EXP_POLISH_ANT"] = row
    _D.CUSTOM_DVE_SPECS["EXP_POLISH_ANT"] = op.spec
    return op


EXP_POLISH = _install_polish()


BF16 = ml_dtypes.bfloat16
FP32 = np.float32

R = 8            # cores
B, L, DIM = 2, 2048, 2048
H, KV, HD = 32, 8, 64
HL = H // R      # 4 local q heads per core
BL = B * L       # 4096
KT = DIM // 128  # 16 contraction tiles
CL = HL * HD     # 256 local q/out dims
SH = BL // R     # 512 seq rows per core in phase 3
NS = BL // 512   # 8 phase-1 seq chunks
ROWS = 260       # a2a shard rows: 256 oT dims + 4 den rows
A2AR = R * ROWS  # 2080

SWAP = [i ^ 1 for i in range(32)]   # pair-swap permutation for rope

_BF = mybir.dt.bfloat16
_F32 = mybir.dt.float32
_I32 = mybir.dt.int32

_cache = {}


def _emit(nc, t):
    mult = mybir.AluOpType.mult
    add = mybir.AluOpType.add
    Exp = mybir.ActivationFunctionType.Exp
    Copy = mybir.ActivationFunctionType.Copy
    tc = t["tc"]

    with tc.tile_pool(name="persist", bufs=1) as P, \
         tc.tile_pool(name="dramp", bufs=1, space="DRAM") as DP:
        wcomb_sb = P.tile([128, KT * 384], _BF, name="wcomb_sb")
        uq_sb = P.tile([128, L], _BF, name="uq_sb")
        wq_sb = P.tile([128, L], _BF, name="wq_sb")
        ukv_sb = P.tile([128, L], _BF, name="ukv_sb")
        wkv_sb = P.tile([128, L], _BF, name="wkv_sb")
        mask_sb = P.tile([128, 128], _BF, name="mask_sb")
        ident_sb = P.tile([128, 128], _BF, name="ident_sb")
        e0_sb = P.tile([2, 128], _BF, name="e0_sb")
        qT = [P.tile([128, BL], _BF, name=f"qT{hp}") for hp in range(2)]
        kv_sb = P.tile([128, BL], _BF, name="kv_sb")
        kdup = P.tile([128, BL], _BF, name="kdup")
        v1 = P.tile([128, 32 * 65], _BF, name="v1")
        ebias_sb = P.tile([128, 1], _F32, name="ebias_sb")
        escale_sb = P.tile([128, 1], _F32, name="escale_sb")
        wot_sb = P.tile([128, KT * DIM], _BF, name="wot_sb")
        denb = P.tile([32, 256], _BF, name="denb")
        rcp32 = P.tile([32, 256], _BF, name="rcp32")
        rcp2 = P.tile([2, 16 * 256], _BF, name="rcp2")

        a2a_in = [DP.tile([A2AR, 256], _BF, tag=f"a2a_in{b}", name=f"a2a_in{b}")
                  for b in range(B)]
        a2a_out = [DP.tile([A2AR, 256], _BF, tag=f"a2a_out{b}",
                           name=f"a2a_out{b}") for b in range(B)]

        # --- constant / weight loads (all host-prepacked [128, F]) --------
        for kt in range(KT):
            nc.sync.dma_start(out=wcomb_sb[:, kt * 384:(kt + 1) * 384],
                              in_=t["wcomb"].ap()[:, kt * 384:(kt + 1) * 384])
        for name, sb in (("uq", uq_sb), ("wq", wq_sb),
                         ("ukv", ukv_sb), ("wkv", wkv_sb)):
            nc.sync.dma_start(out=sb[:], in_=t[name].ap())
        nc.sync.dma_start(out=mask_sb[:], in_=t["mask"].ap())
        nc.sync.dma_start(out=ident_sb[:], in_=t["ident"].ap())
        nc.sync.dma_start(out=e0_sb[:], in_=t["e0"].ap())
        nc.vector.memset(
            v1[:].rearrange("p (t c) -> p t c", c=65)[:, :, 64:65], 1.0
        )
        nc.vector.memset(ebias_sb[:], ACT_EXP_BIAS)
        nc.vector.memset(escale_sb[:], ACT_EXP_SCALE)

        # --- phase 1: transposed qkv projection + rope -------------------
        with tc.tile_pool(name="pp1", bufs=1, space="PSUM") as pp1, \
             tc.tile_pool(name="ppV", bufs=1, space="PSUM") as ppV, \
             tc.tile_pool(name="sp1", bufs=1) as sp1:
            for s in range(NS):
                c0 = (s % 4) * 512       # position within batch
                sc = s * 512             # column in [*, BL] tiles
                xts = []
                for kt in range(KT):
                    xt = sp1.tile([128, 512], _BF, tag="xt", bufs=8,
                                  name=f"xt{s}_{kt}")
                    nc.sync.dma_start(out=xt[:],
                                      in_=t["xt3"].ap()[s * KT + kt])
                    xts.append(xt)
                pss = [pp1.tile([128, 512], _F32, tag="qkv", bufs=6,
                                name=f"qkv{s}_{blk}") for blk in range(3)]
                for kt in range(KT):
                    for blk in range(3):
                        nc.tensor.matmul(
                            pss[blk][:],
                            wcomb_sb[:, kt * 384 + blk * 128:
                                     kt * 384 + (blk + 1) * 128],
                            xts[kt][:],
                            start=(kt == 0), stop=(kt == KT - 1),
                            skip_group_check=True,
                        )
                # rope: dest = (q*u) + pairswap(q*w); v rows pass through
                # the kv tables as identity (u=1, w=0).
                for blk in range(3):
                    u, w = ((uq_sb, wq_sb) if blk < 2 else (ukv_sb, wkv_sb))
                    dest = (qT[blk] if blk < 2 else kv_sb)
                    t1 = sp1.tile([128, 512], _BF, tag="t1", bufs=2,
                                  name=f"t1_{s}_{blk}")
                    t2 = sp1.tile([128, 512], _BF, tag="t2", bufs=2,
                                  name=f"t2_{s}_{blk}")
                    t2s = sp1.tile([128, 512], _BF, tag="t2s", bufs=2,
                                   name=f"t2s_{s}_{blk}")
                    nc.vector.tensor_tensor(
                        t1[:], pss[blk][:], u[:, c0:c0 + 512], mult)
                    nc.vector.tensor_tensor(
                        t2[:], pss[blk][:], w[:, c0:c0 + 512], mult)
                    nc.vector.stream_shuffle(t2s[:], t2[:], SWAP)
                    nc.vector.tensor_tensor(
                        dest[:, sc:sc + 512], t1[:], t2s[:], add)
                # v back to [seq, hd] for the PV stationary
                for tt in range(4):
                    vp = ppV.tile([128, 64], _BF, tag="vp", bufs=2,
                                  name=f"vp{s}_{tt}")
                    nc.tensor.transpose(
                        vp[:], kv_sb[64:128, sc + tt * 128:sc + (tt + 1) * 128],
                        ident_sb[64:128, 64:128])
                    nc.scalar.activation(
                        v1[:, (s * 4 + tt) * 65:(s * 4 + tt) * 65 + 64],
                        vp[:], Copy)
                # duplicate k into rows 64-127 for the row-packed S matmuls
                nc.sync.dma_start(out=kdup[64:128, sc:sc + 512],
                                  in_=kv_sb[0:64, sc:sc + 512])

        # wot loads (needed in phase 3; emitted here so DMA happens in the
        # background during phase 2)
        for q in range(4):
            wq_ = KT * DIM // 4
            nc.sync.dma_start(
                out=wot_sb[:, q * wq_:(q + 1) * wq_],
                in_=t["wot"].ap()[:, q * wq_:(q + 1) * wq_])

        # --- phases 2+3: attention, per-batch all-to-all, out projection ---
        with tc.tile_pool(name="ppS", bufs=1, space="PSUM") as ppS, \
             tc.tile_pool(name="ppO", bufs=1, space="PSUM") as ppO, \
             tc.tile_pool(name="sp2", bufs=1) as sp2, \
             tc.tile_pool(name="sp3", bufs=1) as sp3:

            t["ebias"], t["escale"] = ebias_sb, escale_sb
            exp_ctr = [0]

            def emit_exp(p_bf, p_int, s_ps, lo, hi):
                # base-2 scores (q pre-scaled by log2e/8 in the tables).
                # ~5/8 of pieces: exact exp on ScalarE, scale-matched via
                # bias. 3/8: Schraudolph int32 exp + custom-DVE parabola
                # polish; its global 1/k scale cancels in the softmax.
                exp_ctr[0] += 1
                if (exp_ctr[0] * 5) % 8 < 5:
                    nc.scalar.activation(p_bf[:, lo:hi], s_ps[:, lo:hi], Exp,
                                         bias=t["ebias"][:],
                                         scale=t["escale"][:])
                else:
                    nc.vector.tensor_scalar(
                        p_int[:, lo:hi], s_ps[:, lo:hi], EXP_A1, EXP_B1,
                        mult, add)
                    nc.vector._custom_dve(
                        EXP_POLISH, out=p_bf[:, lo:hi], in0=s_ps[:, lo:hi],
                        in1=p_int[:, lo:hi].bitcast(_F32),
                        s0=MAGIC, s1=INVK)

            def attn_group(b, hp, cp, mask_on_vector):
                """One (batch, head-pair, chunk-pair) of attention work."""
                cpair = (2 * cp, 2 * cp + 1)
                opair = {}
                for c in cpair:
                    opair[c] = ppO.tile([65, 1024], _F32, tag="o", bufs=2,
                                        name=f"o{b}_{hp}_{c}")
                njt = 4 * cpair[1] + 4
                for jt in range(njt):
                    jcol = b * L + jt * 128
                    for c in cpair:
                        if jt >= 4 * c + 4:
                            continue
                        a = max(0, jt * 128 - c * 512)
                        iw = b * L + c * 512
                        n = 512 - a
                        s_t = ppS.tile([128, 1024], _F32, tag="S", bufs=2,
                                       name=f"s{b}_{hp}_{c}_{jt}")
                        nc.tensor.matmul(
                            s_t[:, a:512],
                            kv_sb[0:64, jcol:jcol + 128],
                            qT[hp][0:64, iw + a:iw + 512],
                            start=True, stop=True,
                        )
                        nc.tensor.matmul(
                            s_t[:, 512 + a:1024],
                            kdup[64:128, jcol:jcol + 128],
                            qT[hp][64:128, iw + a:iw + 512],
                            start=True, stop=True,
                        )
                        p_int = sp2.tile([128, 1024], _I32, tag="PI", bufs=2,
                                         name=f"pi{b}_{hp}_{c}_{jt}")
                        p_bf = sp2.tile([128, 1024], _BF, tag="P", bufs=4,
                                        name=f"p{b}_{hp}_{c}_{jt}")
                        if a == 0:
                            emit_exp(p_bf, p_int, s_t, 0, 1024)
                        else:
                            emit_exp(p_bf, p_int, s_t, a, 512)
                            emit_exp(p_bf, p_int, s_t, 512 + a, 1024)
                        if jt >= 4 * c:
                            # causal mask on the two diagonal 128-blocks
                            pb3 = p_bf[:].rearrange("p (h c) -> p h c", h=2)
                            meng = nc.vector if mask_on_vector else nc.gpsimd
                            meng.tensor_tensor(
                                pb3[:, :, a:a + 128], pb3[:, :, a:a + 128],
                                mask_sb[:].unsqueeze(1).broadcast_to(
                                    [128, 2, 128]),
                                mult)
                        vsl = v1[:, (b * 16 + jt) * 65:(b * 16 + jt) * 65 + 65]
                        nc.tensor.matmul(
                            opair[c][0:65, a:512], vsl, p_bf[:, a:512],
                            start=(jt == 0), stop=(jt == 4 * c + 3),
                            skip_group_check=True,
                        )
                        nc.tensor.matmul(
                            opair[c][0:65, 512 + a:1024], vsl,
                            p_bf[:, 512 + a:1024],
                            start=(jt == 0), stop=(jt == 4 * c + 3),
                            skip_group_check=True,
                        )
                # evacuate + stage for the all-to-all
                for c in cpair:
                    stage = sp2.tile([65, 1024], _BF, tag="stg", bufs=3,
                                     name=f"stg{b}_{hp}_{c}")
                    nc.scalar.activation(stage[:], opair[c][:], Copy)
                    for h2 in range(2):
                        h = hp * 2 + h2
                        for j2 in range(2):
                            j = 2 * c + j2
                            src0 = h2 * 512 + j2 * 256
                            nc.sync.dma_start(
                                out=a2a_in[b][j * ROWS + h * 64:
                                              j * ROWS + (h + 1) * 64, :],
                                in_=stage[0:64, src0:src0 + 256])
                            nc.sync.dma_start(
                                out=a2a_in[b][j * ROWS + 256 + h:
                                              j * ROWS + 257 + h, :],
                                in_=stage[64:65, src0:src0 + 256])

            def p3_prep(b):
                # gather den rows (glo-major), reciprocal, remap, then
                # normalize each lhs tile with a K=2 broadcast matmul
                for i in range(R):
                    r0 = i * ROWS + 256
                    nc.sync.dma_start(out=denb[2 * i:2 * i + 2, :],
                                      in_=a2a_out[b][r0:r0 + 4:2, :])
                    nc.sync.dma_start(out=denb[16 + 2 * i:18 + 2 * i, :],
                                      in_=a2a_out[b][r0 + 1:r0 + 4:2, :])
                with nc.allow_low_precision(reason="bf16 1/den; 0.4% ok"):
                    nc.vector.reciprocal(rcp32[:], denb[:])
                for glo in range(2):
                    nc.sync.dma_start(out=rcp2[glo:glo + 1, :],
                                      in_=rcp32[glo * 16:(glo + 1) * 16, :])
                lhs = []
                for ct in range(KT):
                    lt = sp3.tile([128, 256], _BF, tag="lhs", bufs=17,
                                  name=f"lhs{b}_{ct}")
                    i = ct // 2
                    nc.sync.dma_start(
                        out=lt[:],
                        in_=a2a_out[b][i * ROWS + (ct % 2) * 128:
                                       i * ROWS + (ct % 2) * 128 + 128, :])
                    rb = ppS.tile([128, 256], _F32, tag="S", bufs=2,
                                  name=f"rb{b}_{ct}")
                    nc.tensor.matmul(rb[:], e0_sb[0:2, :],
                                     rcp2[0:2, ct * 256:(ct + 1) * 256],
                                     start=True, stop=True)
                    nc.vector.tensor_tensor(lt[:], lt[:], rb[:], mult)
                    lhs.append(lt)
                return lhs

            def p3_mms(b, lhs, it2):
                for n in range(4):
                    y_ps = ppS.tile([128, 512], _F32, tag="S", bufs=2,
                                    name=f"y{b}_{it2}_{n}")
                    for ct in range(KT):
                        nc.tensor.matmul(
                            y_ps[:],
                            lhs[ct][:, it2 * 128:(it2 + 1) * 128],
                            wot_sb[:, ct * DIM + n * 512:
                                   ct * DIM + n * 512 + 512],
                            start=(ct == 0), stop=(ct == KT - 1))
                    y_sb = sp3.tile([128, 512], _F32, tag="ysb", bufs=2,
                                    name=f"ysb{b}_{it2}_{n}")
                    if n % 2 == 0:
                        nc.scalar.activation(y_sb[:], y_ps[:], Copy)
                    else:
                        nc.vector.tensor_scalar(y_sb[:], y_ps[:], 1.0, 0.0,
                                                mult, add)
                    nc.sync.dma_start(
                        out=t["out"][b * 256 + it2 * 128:
                                     b * 256 + (it2 + 1) * 128,
                                     n * 512:(n + 1) * 512],
                        in_=y_sb[:])

            # batch 0 attention
            for hp in range(2):
                for cp in range(2):
                    attn_group(0, hp, cp, mask_on_vector=False)
            nc.gpsimd.collective_compute(
                "AllToAll", mybir.AluOpType.bypass,
                replica_groups=[list(range(R))],
                ins=[a2a_in[0][:].opt()],
                outs=[a2a_out[0][:].opt()],
            )
            # batch 1 attention with batch-0 projection interleaved;
            # masks early in batch 1 ride on DVE (gpsimd queue holds the
            # collective wait)
            attn_group(1, 0, 0, mask_on_vector=True)
            attn_group(1, 0, 1, mask_on_vector=True)
            lhs0 = p3_prep(0)
            attn_group(1, 1, 0, mask_on_vector=False)
            p3_mms(0, lhs0, 0)
            attn_group(1, 1, 1, mask_on_vector=False)
            p3_mms(0, lhs0, 1)
            nc.gpsimd.collective_compute(
                "AllToAll", mybir.AluOpType.bypass,
                replica_groups=[list(range(R))],
                ins=[a2a_in[1][:].opt()],
                outs=[a2a_out[1][:].opt()],
            )
            lhs1 = p3_prep(1)
            p3_mms(1, lhs1, 0)
            p3_mms(1, lhs1, 1)


def _build():
    if "nc" in _cache:
        return _cache["nc"]
    nc = bacc.Bacc("TRN2", target_bir_lowering=False, debug=False,
                   enable_asserts=False, num_devices=R)
    t = {}
    t["xt3"] = nc.dram_tensor("xt3", [NS * KT, 128, 512], _BF,
                              kind="ExternalInput")
    t["wcomb"] = nc.dram_tensor("wcomb", [128, KT * 384], _BF,
                                kind="ExternalInput")
    for name in ("uq", "wq", "ukv", "wkv"):
        t[name] = nc.dram_tensor(name, [128, L], _BF, kind="ExternalInput")
    t["mask"] = nc.dram_tensor("mask", [128, 128], _BF, kind="ExternalInput")
    t["ident"] = nc.dram_tensor("ident", [128, 128], _BF, kind="ExternalInput")
    t["e0"] = nc.dram_tensor("e0", [2, 128], _BF, kind="ExternalInput")
    t["wot"] = nc.dram_tensor("wot", [128, KT * DIM], _BF,
                              kind="ExternalInput")
    t["out"] = nc.dram_tensor("out", [SH, DIM], _F32, kind="ExternalOutput")

    with tile.TileContext(nc) as tc:
        t["tc"] = tc
        _emit(nc, t)
    nc.compile()
    _cache["nc"] = nc
    return nc


def _prep_inputs(x, freqs_cis, wq, wk, wv, wo):
    x = np.asarray(x, dtype=FP32)
    freqs_cis = np.asarray(freqs_cis, dtype=FP32)
    wq = np.asarray(wq, dtype=FP32)
    wk = np.asarray(wk, dtype=FP32)
    wv = np.asarray(wv, dtype=FP32)
    wo = np.asarray(wo, dtype=FP32)

    xf = x.reshape(BL, DIM)
    # x^T tiles: [s-chunk, kt, 128 dim, 512 seq]
    xt3 = np.ascontiguousarray(
        xf.reshape(NS, 512, KT, 128).transpose(0, 2, 3, 1)
    ).reshape(NS * KT, 128, 512).astype(BF16)

    def pack128(a3):
        # [NT, 128, C] -> [128, NT*C] with row p = concat over tiles
        n, _, c = a3.shape
        return np.ascontiguousarray(a3.transpose(1, 0, 2).reshape(128, n * c))

    # rope tables in [dims, seq] layout. row p covers head-local dim
    # d = p % 64 (pairs interleaved); cos/sin vary along seq (free dim).
    cos = np.cos(np.arange(L)[:, None] *
                 (1.0 / (10000.0 ** (np.arange(0, HD, 2) / HD)))[None, :])
    sin = np.sin(np.arange(L)[:, None] *
                 (1.0 / (10000.0 ** (np.arange(0, HD, 2) / HD)))[None, :])
    cosT = cos.T.astype(FP32)   # [32, L]
    sinT = sin.T.astype(FP32)
    qs = 0.125 * LOG2E
    uq = np.empty((128, L), dtype=FP32)
    wq_t = np.empty((128, L), dtype=FP32)
    for p in range(128):
        d = p % 64
        f = d // 2
        uq[p] = cosT[f] * qs
        wq_t[p] = (sinT[f] if d % 2 == 0 else -sinT[f]) * qs
    ukv = np.empty((128, L), dtype=FP32)
    wkv = np.empty((128, L), dtype=FP32)
    for p in range(64):
        f = p // 2
        ukv[p] = cosT[f]
        wkv[p] = sinT[f] if p % 2 == 0 else -sinT[f]
    ukv[64:] = 1.0
    wkv[64:] = 0.0

    mask = np.triu(np.ones((128, 128), dtype=FP32)).astype(BF16)
    ident = np.eye(128, dtype=FP32).astype(BF16)
    e0 = np.zeros((2, 128), dtype=FP32)
    e0[0, 0:64] = 1.0
    e0[1, 64:128] = 1.0
    wot = pack128(wo.T.reshape(KT, 128, DIM)).astype(BF16)

    in_maps = []
    for r in range(R):
        wq_sh = wq[r * CL:(r + 1) * CL]          # [256, 2048]
        wk_sh = wk[r * HD:(r + 1) * HD]          # [64, 2048]
        wv_sh = wv[r * HD:(r + 1) * HD]
        wcomb = np.concatenate([wq_sh.T, wk_sh.T, wv_sh.T], axis=1)  # [2048, 384]
        wcomb = pack128(wcomb.reshape(KT, 128, 384)).astype(BF16)
        in_maps.append({
            "xt3": xt3, "wcomb": wcomb,
            "uq": uq.astype(BF16), "wq": wq_t.astype(BF16),
            "ukv": ukv.astype(BF16), "wkv": wkv.astype(BF16),
            "mask": mask, "ident": ident, "e0": e0.astype(BF16),
            "wot": wot,
        })
    return in_maps


def run(inputs, trace=False, trace_cores=None):
    nc = _build()
    in_maps = _prep_inputs(**inputs)
    res = run_bass_kernel_spmd(
        nc, in_maps, core_ids=list(range(R)), trace=trace,
        trace_cores=trace_cores,
    )
    shards = [np.asarray(res.results[r]["out"], dtype=FP32) for r in range(R)]
    y = np.empty((BL, DIM), dtype=FP32)
    for r in range(R):
        y[256 * r:256 * (r + 1)] = shards[r][0:256]
        y[L + 256 * r:L + 256 * (r + 1)] = shards[r][256:512]
    return y.reshape(B, L, DIM), res


def kernel(x, freqs_cis, wq, wk, wv, wo):
    y, _ = run(dict(x=x, freqs_cis=freqs_cis, wq=wq, wk=wk, wv=wv, wo=wo))
    return y


# revision 11
# speedup vs baseline: 1.1400x; 1.1400x over previous
"""Distributed Trainium2 kernel for GQA causal attention (B=2, L=2048, DIM=2048,
H=32 q-heads, KV=8 kv-heads, HD=64), tensor-parallel over heads across 8 cores.

v2 layout: everything stays transposed ([dims, seq]) end-to-end.

Per-core pipeline (core r owns q heads 4r..4r+3 and kv head r):
  P1: qkvT = wcomb^T-stationary matmuls streaming x^T (N=512) -> q/k emerge
      directly in [head_dim, seq] layout (no PE transposes); RoPE via 4 DVE
      ops (2 mults against packed cos/sin tables, stream_shuffle pair-swap,
      add); v recovered to [seq, hd] by 32 small PE transposes.
  P2: scores computed transposed (S^T[j,i]) with the two heads of a pair
      packed into the two 64-row halves of the PE array (GQA: they share the
      kv head; stationary is the kv block duplicated to both halves) -> one
      512-cycle stream yields two heads' scores. exp split ACT/DVE
      (Schraudolph+polish custom op), denominator via ones-column of V.
      jt-outer / chunk-inner loop amortizes weight loads.
  P2.5: per-batch AllToAll moves o^T blocks [dims, seq-shard] plus raw
      denominator rows; output lands as [all 2048 dims, my 256 seq] directly
      usable as projection lhsT (no transposes anywhere).
  P3: normalize lhs tiles with a K=2 broadcast matmul of 1/den, then full
      output projection for this core's seq shard.
Host: concatenates the 8 row-shards.
"""

import sys

sys.path.insert(0, "/opt/trn_rl_repo")

import numpy as np
import ml_dtypes

from concourse import bass, bacc, mybir, tile
from concourse.bass_utils import run_bass_kernel_spmd

# ---- custom DVE op: Schraudolph exp + parabola mantissa polish ------------
# out = Src1 * ((f^2 - |f|) + 1/k), f = Src0 - rint(Src0) via the magic-add;
# since f^2-|f| = fhat^2-fhat for fhat = frac(Src0), this multiplies the
# Schraudolph value y = 2^x*(1+fhat)/2^fhat by (1 + k*(fhat^2-fhat))/k, a
# two-parameter fit of 2^fhat/(1+fhat) (0.4% rms). The global 1/k scale
# cancels in the softmax ratio; ScalarE pieces match it via an exp bias.
from concourse import dve_ops as _D
from concourse.dve_spec import (AluOp as _AluOp, Bin as _Bin, Spec as _Spec,
                                Src0 as _Src0, Src1 as _Src1, C0 as _C0,
                                C1 as _C1, Zero as _Zero, sq as _sq,
                                lower as _dve_lower)
from concourse.dve_uop import DveOpSpec as _DveOpSpec

MAGIC = 12582912.0
KPAR = 0.2150
INVK = 1.0 / KPAR
ASCALE = 0.996908
EXP_A1 = float(np.float32(2 ** 23))
EXP_B1 = float(np.float32(127 * 2 ** 23) + np.float32(2 ** 23 * np.log2(ASCALE)))
LOG2E = float(np.float64(1.0) / np.log(2.0))
ACT_EXP_SCALE = float(np.log(2.0))
ACT_EXP_BIAS = float(np.log(INVK))


def _polish_ref(in0, in1, s0, s1, imm2):
    f = (in0 - np.rint(in0)).astype(np.float32)
    return ((f * f - np.abs(f)) + np.float32(s1)) * in1


def _install_polish():
    for op in _D.OPS:
        if op.name == "EXP_POLISH_ANT":
            return op
    f = _Bin(_AluOp.SUBTRACT, _Src0,
             _Bin(_AluOp.SUBTRACT, _Bin(_AluOp.ADD, _Src0, _C0), _C0))
    g = _Bin(_AluOp.SUBTRACT, _sq(f), _Bin(_AluOp.ABSOLUTE_DIFF, f, _Zero))
    body = _Bin(_AluOp.MULTIPLY, _Bin(_AluOp.ADD, g, _C1), _Src1)
    row = _D._CUSTOM_DVE_ROW_BASE + len(_D.OPS)
    spec = _Spec(body=body, reference=_polish_ref)
    sha = _DveOpSpec(name="EXP_POLISH_ANT", opcode=row,
                     uops=_dve_lower(spec, ver="v3"), rd1_en=True).sha("v3")
    op = _D.DveOp("EXP_POLISH_ANT", spec, subdim=False,
                  uops_sha={"v3": sha, "v4": sha})
    _D.OPS.append(op)
    _D._SUB_OPCODE_FOR_NAME["EXP_POLISH_ANT"] = row
    _D.CUSTOM_DVE_SPECS["EXP_POLISH_ANT"] = op.spec
    return op


EXP_POLISH = _install_polish()


BF16 = ml_dtypes.bfloat16
FP32 = np.float32

R = 8            # cores
B, L, DIM = 2, 2048, 2048
H, KV, HD = 32, 8, 64
HL = H // R      # 4 local q heads per core
BL = B * L       # 4096
KT = DIM // 128  # 16 contraction tiles
CL = HL * HD     # 256 local q/out dims
SH = BL // R     # 512 seq rows per core in phase 3
NS = BL // 512   # 8 phase-1 seq chunks
ROWS = 260       # a2a shard rows: 256 oT dims + 4 den rows
A2AR = R * ROWS  # 2080

SWAP = [i ^ 1 for i in range(32)]   # pair-swap permutation for rope

_BF = mybir.dt.bfloat16
_F32 = mybir.dt.float32
_I32 = mybir.dt.int32

_cache = {}


def _emit(nc, t):
    mult = mybir.AluOpType.mult
    add = mybir.AluOpType.add
    Exp = mybir.ActivationFunctionType.Exp
    Copy = mybir.ActivationFunctionType.Copy
    tc = t["tc"]

    with tc.tile_pool(name="persist", bufs=1) as P, \
         tc.tile_pool(name="dramp", bufs=1, space="DRAM") as DP:
        wcomb_sb = P.tile([128, KT * 384], _BF, name="wcomb_sb")
        uq_sb = P.tile([128, L], _BF, name="uq_sb")
        wq_sb = P.tile([128, L], _BF, name="wq_sb")
        ukv_sb = P.tile([128, L], _BF, name="ukv_sb")
        wkv_sb = P.tile([128, L], _BF, name="wkv_sb")
        mask_sb = P.tile([128, 128], _BF, name="mask_sb")
        ident_sb = P.tile([128, 128], _BF, name="ident_sb")
        e0_sb = P.tile([2, 128], _BF, name="e0_sb")
        qT = [P.tile([128, BL], _BF, name=f"qT{hp}") for hp in range(2)]
        kv_sb = P.tile([128, BL], _BF, name="kv_sb")
        kdup = P.tile([128, BL], _BF, name="kdup")
        v1 = P.tile([128, 32 * 65], _BF, name="v1")
        ebias_sb = P.tile([128, 1], _F32, name="ebias_sb")
        escale_sb = P.tile([128, 1], _F32, name="escale_sb")
        wot_sb = P.tile([128, KT * DIM], _BF, name="wot_sb")
        denb = P.tile([32, 256], _BF, name="denb")
        rcp32 = P.tile([32, 256], _BF, name="rcp32")
        rcp2 = P.tile([2, 16 * 256], _BF, name="rcp2")

        a2a_in = [DP.tile([A2AR, 256], _BF, tag=f"a2a_in{b}", name=f"a2a_in{b}")
                  for b in range(B)]
        a2a_out = [DP.tile([A2AR, 256], _BF, tag=f"a2a_out{b}",
                           name=f"a2a_out{b}") for b in range(B)]

        # --- constant / weight loads (all host-prepacked [128, F]) --------
        # (wcomb is interleaved with the first xt chunk inside the phase-1
        # loop; the rope tables load after the first matmuls are emitted so
        # the DMA queues prioritize what gates the PE.)
        nc.vector.memset(
            v1[:].rearrange("p (t c) -> p t c", c=65)[:, :, 64:65], 1.0
        )
        nc.vector.memset(ebias_sb[:], ACT_EXP_BIAS)
        nc.vector.memset(escale_sb[:], ACT_EXP_SCALE)

        # --- phase 1: transposed qkv projection + rope -------------------
        with tc.tile_pool(name="pp1", bufs=1, space="PSUM") as pp1, \
             tc.tile_pool(name="ppV", bufs=1, space="PSUM") as ppV, \
             tc.tile_pool(name="sp1", bufs=1) as sp1:
            for s in range(NS):
                c0 = (s % 4) * 512       # position within batch
                sc = s * 512             # column in [*, BL] tiles
                xts = []
                for kt in range(KT):
                    if s == 0:
                        nc.sync.dma_start(
                            out=wcomb_sb[:, kt * 384:(kt + 1) * 384],
                            in_=t["wcomb"].ap()[:, kt * 384:(kt + 1) * 384])
                    xt = sp1.tile([128, 512], _BF, tag="xt", bufs=8,
                                  name=f"xt{s}_{kt}")
                    nc.sync.dma_start(out=xt[:],
                                      in_=t["xt3"].ap()[s * KT + kt])
                    xts.append(xt)
                pss = [pp1.tile([128, 512], _F32, tag="qkv", bufs=6,
                                name=f"qkv{s}_{blk}") for blk in range(3)]
                for kt in range(KT):
                    for blk in range(3):
                        nc.tensor.matmul(
                            pss[blk][:],
                            wcomb_sb[:, kt * 384 + blk * 128:
                                     kt * 384 + (blk + 1) * 128],
                            xts[kt][:],
                            start=(kt == 0), stop=(kt == KT - 1),
                            skip_group_check=True,
                        )
                if s == 0:
                    for name, sb in (("uq", uq_sb), ("wq", wq_sb),
                                     ("ukv", ukv_sb), ("wkv", wkv_sb)):
                        nc.sync.dma_start(out=sb[:], in_=t[name].ap())
                    nc.sync.dma_start(out=mask_sb[:], in_=t["mask"].ap())
                    nc.sync.dma_start(out=ident_sb[:], in_=t["ident"].ap())
                    nc.sync.dma_start(out=e0_sb[:], in_=t["e0"].ap())
                # rope: dest = (q*u) + pairswap(q*w); v rows pass through
                # the kv tables as identity (u=1, w=0).
                for blk in range(3):
                    u, w = ((uq_sb, wq_sb) if blk < 2 else (ukv_sb, wkv_sb))
                    dest = (qT[blk] if blk < 2 else kv_sb)
                    t1 = sp1.tile([128, 512], _BF, tag="t1", bufs=2,
                                  name=f"t1_{s}_{blk}")
                    t2 = sp1.tile([128, 512], _BF, tag="t2", bufs=2,
                                  name=f"t2_{s}_{blk}")
                    t2s = sp1.tile([128, 512], _BF, tag="t2s", bufs=2,
                                   name=f"t2s_{s}_{blk}")
                    nc.vector.tensor_tensor(
                        t1[:], pss[blk][:], u[:, c0:c0 + 512], mult)
                    nc.vector.tensor_tensor(
                        t2[:], pss[blk][:], w[:, c0:c0 + 512], mult)
                    nc.vector.stream_shuffle(t2s[:], t2[:], SWAP)
                    nc.vector.tensor_tensor(
                        dest[:, sc:sc + 512], t1[:], t2s[:], add)
                # v back to [seq, hd] for the PV stationary
                for tt in range(4):
                    vp = ppV.tile([128, 64], _BF, tag="vp", bufs=2,
                                  name=f"vp{s}_{tt}")
                    nc.tensor.transpose(
                        vp[:], kv_sb[64:128, sc + tt * 128:sc + (tt + 1) * 128],
                        ident_sb[64:128, 64:128])
                    nc.scalar.activation(
                        v1[:, (s * 4 + tt) * 65:(s * 4 + tt) * 65 + 64],
                        vp[:], Copy)
                # duplicate k into rows 64-127 for the row-packed S matmuls
                nc.sync.dma_start(out=kdup[64:128, sc:sc + 512],
                                  in_=kv_sb[0:64, sc:sc + 512])

        # wot loads (needed in phase 3; emitted here so DMA happens in the
        # background during phase 2)
        for q in range(4):
            wq_ = KT * DIM // 4
            nc.sync.dma_start(
                out=wot_sb[:, q * wq_:(q + 1) * wq_],
                in_=t["wot"].ap()[:, q * wq_:(q + 1) * wq_])

        # --- phases 2+3: attention, per-batch all-to-all, out projection ---
        with tc.tile_pool(name="ppS", bufs=1, space="PSUM") as ppS, \
             tc.tile_pool(name="ppO", bufs=1, space="PSUM") as ppO, \
             tc.tile_pool(name="sp2", bufs=1) as sp2, \
             tc.tile_pool(name="sp3", bufs=1) as sp3:

            t["ebias"], t["escale"] = ebias_sb, escale_sb
            exp_ctr = [0]

            def emit_exp(p_bf, p_int, s_ps, lo, hi):
                # base-2 scores (q pre-scaled by log2e/8 in the tables).
                # ~5/8 of pieces: exact exp on ScalarE, scale-matched via
                # bias. 3/8: Schraudolph int32 exp + custom-DVE parabola
                # polish; its global 1/k scale cancels in the softmax.
                exp_ctr[0] += 1
                if exp_ctr[0] % 3 < 2:
                    nc.scalar.activation(p_bf[:, lo:hi], s_ps[:, lo:hi], Exp,
                                         bias=t["ebias"][:],
                                         scale=t["escale"][:])
                else:
                    nc.vector.tensor_scalar(
                        p_int[:, lo:hi], s_ps[:, lo:hi], EXP_A1, EXP_B1,
                        mult, add)
                    nc.vector._custom_dve(
                        EXP_POLISH, out=p_bf[:, lo:hi], in0=s_ps[:, lo:hi],
                        in1=p_int[:, lo:hi].bitcast(_F32),
                        s0=MAGIC, s1=INVK)

            def attn_group(b, hp, cp, mask_on_vector):
                """One (batch, head-pair, chunk-pair) of attention work."""
                cpair = (2 * cp, 2 * cp + 1)
                opair = {}
                for c in cpair:
                    opair[c] = ppO.tile([65, 1024], _F32, tag="o", bufs=2,
                                        name=f"o{b}_{hp}_{c}")
                njt = 4 * cpair[1] + 4

                def emit_pv(items):
                    # PV runs one jt behind S+exp so the PE never waits on
                    # the in-flight exp
                    for (c, jt, a, p_bf) in items:
                        vsl = v1[:, (b * 16 + jt) * 65:
                                 (b * 16 + jt) * 65 + 65]
                        nc.tensor.matmul(
                            opair[c][0:65, a:512], vsl, p_bf[:, a:512],
                            start=(jt == 0), stop=(jt == 4 * c + 3),
                            skip_group_check=True,
                        )
                        nc.tensor.matmul(
                            opair[c][0:65, 512 + a:1024], vsl,
                            p_bf[:, 512 + a:1024],
                            start=(jt == 0), stop=(jt == 4 * c + 3),
                            skip_group_check=True,
                        )

                prev = []
                for jt in range(njt):
                    jcol = b * L + jt * 128
                    cur = []
                    for c in cpair:
                        if jt >= 4 * c + 4:
                            continue
                        a = max(0, jt * 128 - c * 512)
                        iw = b * L + c * 512
                        s_t = ppS.tile([128, 1024], _F32, tag="S", bufs=2,
                                       name=f"s{b}_{hp}_{c}_{jt}")
                        nc.tensor.matmul(
                            s_t[:, a:512],
                            kv_sb[0:64, jcol:jcol + 128],
                            qT[hp][0:64, iw + a:iw + 512],
                            start=True, stop=True,
                        )
                        nc.tensor.matmul(
                            s_t[:, 512 + a:1024],
                            kdup[64:128, jcol:jcol + 128],
                            qT[hp][64:128, iw + a:iw + 512],
                            start=True, stop=True,
                        )
                        p_int = sp2.tile([128, 1024], _I32, tag="PI", bufs=2,
                                         name=f"pi{b}_{hp}_{c}_{jt}")
                        p_bf = sp2.tile([128, 1024], _BF, tag="P", bufs=6,
                                        name=f"p{b}_{hp}_{c}_{jt}")
                        if a == 0:
                            emit_exp(p_bf, p_int, s_t, 0, 1024)
                        else:
                            emit_exp(p_bf, p_int, s_t, a, 512)
                            emit_exp(p_bf, p_int, s_t, 512 + a, 1024)
                        if jt >= 4 * c:
                            # causal mask on the two diagonal 128-blocks
                            pb3 = p_bf[:].rearrange("p (h c) -> p h c", h=2)
                            meng = nc.vector if mask_on_vector else nc.gpsimd
                            meng.tensor_tensor(
                                pb3[:, :, a:a + 128], pb3[:, :, a:a + 128],
                                mask_sb[:].unsqueeze(1).broadcast_to(
                                    [128, 2, 128]),
                                mult)
                        cur.append((c, jt, a, p_bf))
                    emit_pv(prev)
                    prev = cur
                emit_pv(prev)
                # evacuate + stage for the all-to-all
                for c in cpair:
                    stage = sp2.tile([65, 1024], _BF, tag="stg", bufs=3,
                                     name=f"stg{b}_{hp}_{c}")
                    nc.scalar.activation(stage[:], opair[c][:], Copy)
                    for h2 in range(2):
                        h = hp * 2 + h2
                        for j2 in range(2):
                            j = 2 * c + j2
                            src0 = h2 * 512 + j2 * 256
                            nc.sync.dma_start(
                                out=a2a_in[b][j * ROWS + h * 64:
                                              j * ROWS + (h + 1) * 64, :],
                                in_=stage[0:64, src0:src0 + 256])
                            nc.sync.dma_start(
                                out=a2a_in[b][j * ROWS + 256 + h:
                                              j * ROWS + 257 + h, :],
                                in_=stage[64:65, src0:src0 + 256])

            def p3_prep(b):
                # gather den rows (glo-major), reciprocal, remap, then
                # normalize each lhs tile with a K=2 broadcast matmul
                for i in range(R):
                    r0 = i * ROWS + 256
                    nc.sync.dma_start(out=denb[2 * i:2 * i + 2, :],
                                      in_=a2a_out[b][r0:r0 + 4:2, :])
                    nc.sync.dma_start(out=denb[16 + 2 * i:18 + 2 * i, :],
                                      in_=a2a_out[b][r0 + 1:r0 + 4:2, :])
                with nc.allow_low_precision(reason="bf16 1/den; 0.4% ok"):
                    nc.vector.reciprocal(rcp32[:], denb[:])
                for glo in range(2):
                    nc.sync.dma_start(out=rcp2[glo:glo + 1, :],
                                      in_=rcp32[glo * 16:(glo + 1) * 16, :])
                lhs = []
                for ct in range(KT):
                    lt = sp3.tile([128, 256], _BF, tag="lhs", bufs=17,
                                  name=f"lhs{b}_{ct}")
                    i = ct // 2
                    nc.sync.dma_start(
                        out=lt[:],
                        in_=a2a_out[b][i * ROWS + (ct % 2) * 128:
                                       i * ROWS + (ct % 2) * 128 + 128, :])
                    rb = ppS.tile([128, 256], _F32, tag="S", bufs=2,
                                  name=f"rb{b}_{ct}")
                    nc.tensor.matmul(rb[:], e0_sb[0:2, :],
                                     rcp2[0:2, ct * 256:(ct + 1) * 256],
                                     start=True, stop=True)
                    nc.vector.tensor_tensor(lt[:], lt[:], rb[:], mult)
                    lhs.append(lt)
                return lhs

            def p3_mms(b, lhs, it2):
                for n in range(4):
                    y_ps = ppS.tile([128, 512], _F32, tag="S", bufs=2,
                                    name=f"y{b}_{it2}_{n}")
                    for ct in range(KT):
                        nc.tensor.matmul(
                            y_ps[:],
                            lhs[ct][:, it2 * 128:(it2 + 1) * 128],
                            wot_sb[:, ct * DIM + n * 512:
                                   ct * DIM + n * 512 + 512],
                            start=(ct == 0), stop=(ct == KT - 1))
                    y_sb = sp3.tile([128, 512], _F32, tag="ysb", bufs=2,
                                    name=f"ysb{b}_{it2}_{n}")
                    if n % 2 == 0:
                        nc.scalar.activation(y_sb[:], y_ps[:], Copy)
                    else:
                        nc.vector.tensor_scalar(y_sb[:], y_ps[:], 1.0, 0.0,
                                                mult, add)
                    nc.sync.dma_start(
                        out=t["out"][b * 256 + it2 * 128:
                                     b * 256 + (it2 + 1) * 128,
                                     n * 512:(n + 1) * 512],
                        in_=y_sb[:])

            # batch 0 attention
            for hp in range(2):
                for cp in range(2):
                    attn_group(0, hp, cp, mask_on_vector=False)
            nc.gpsimd.collective_compute(
                "AllToAll", mybir.AluOpType.bypass,
                replica_groups=[list(range(R))],
                ins=[a2a_in[0][:].opt()],
                outs=[a2a_out[0][:].opt()],
            )
            # batch 1 attention with batch-0 projection interleaved;
            # masks early in batch 1 ride on DVE (gpsimd queue holds the
            # collective wait)
            attn_group(1, 0, 0, mask_on_vector=True)
            attn_group(1, 0, 1, mask_on_vector=True)
            lhs0 = p3_prep(0)
            attn_group(1, 1, 0, mask_on_vector=False)
            p3_mms(0, lhs0, 0)
            attn_group(1, 1, 1, mask_on_vector=False)
            p3_mms(0, lhs0, 1)
            nc.gpsimd.collective_compute(
                "AllToAll", mybir.AluOpType.bypass,
                replica_groups=[list(range(R))],
                ins=[a2a_in[1][:].opt()],
                outs=[a2a_out[1][:].opt()],
            )
            lhs1 = p3_prep(1)
            p3_mms(1, lhs1, 0)
            p3_mms(1, lhs1, 1)


def _build():
    if "nc" in _cache:
        return _cache["nc"]
    nc = bacc.Bacc("TRN2", target_bir_lowering=False, debug=False,
                   enable_asserts=False, num_devices=R)
    t = {}
    t["xt3"] = nc.dram_tensor("xt3", [NS * KT, 128, 512], _BF,
                              kind="ExternalInput")
    t["wcomb"] = nc.dram_tensor("wcomb", [128, KT * 384], _BF,
                                kind="ExternalInput")
    for name in ("uq", "wq", "ukv", "wkv"):
        t[name] = nc.dram_tensor(name, [128, L], _BF, kind="ExternalInput")
    t["mask"] = nc.dram_tensor("mask", [128, 128], _BF, kind="ExternalInput")
    t["ident"] = nc.dram_tensor("ident", [128, 128], _BF, kind="ExternalInput")
    t["e0"] = nc.dram_tensor("e0", [2, 128], _BF, kind="ExternalInput")
    t["wot"] = nc.dram_tensor("wot", [128, KT * DIM], _BF,
                              kind="ExternalInput")
    t["out"] = nc.dram_tensor("out", [SH, DIM], _F32, kind="ExternalOutput")

    with tile.TileContext(nc) as tc:
        t["tc"] = tc
        _emit(nc, t)
    nc.compile()
    _cache["nc"] = nc
    return nc


def _prep_inputs(x, freqs_cis, wq, wk, wv, wo):
    x = np.asarray(x, dtype=FP32)
    freqs_cis = np.asarray(freqs_cis, dtype=FP32)
    wq = np.asarray(wq, dtype=FP32)
    wk = np.asarray(wk, dtype=FP32)
    wv = np.asarray(wv, dtype=FP32)
    wo = np.asarray(wo, dtype=FP32)

    xf = x.reshape(BL, DIM)
    # x^T tiles: [s-chunk, kt, 128 dim, 512 seq]
    xt3 = np.ascontiguousarray(
        xf.reshape(NS, 512, KT, 128).transpose(0, 2, 3, 1)
    ).reshape(NS * KT, 128, 512).astype(BF16)

    def pack128(a3):
        # [NT, 128, C] -> [128, NT*C] with row p = concat over tiles
        n, _, c = a3.shape
        return np.ascontiguousarray(a3.transpose(1, 0, 2).reshape(128, n * c))

    # rope tables in [dims, seq] layout. row p covers head-local dim
    # d = p % 64 (pairs interleaved); cos/sin vary along seq (free dim).
    cos = np.cos(np.arange(L)[:, None] *
                 (1.0 / (10000.0 ** (np.arange(0, HD, 2) / HD)))[None, :])
    sin = np.sin(np.arange(L)[:, None] *
                 (1.0 / (10000.0 ** (np.arange(0, HD, 2) / HD)))[None, :])
    cosT = cos.T.astype(FP32)   # [32, L]
    sinT = sin.T.astype(FP32)
    qs = 0.125 * LOG2E
    uq = np.empty((128, L), dtype=FP32)
    wq_t = np.empty((128, L), dtype=FP32)
    for p in range(128):
        d = p % 64
        f = d // 2
        uq[p] = cosT[f] * qs
        wq_t[p] = (sinT[f] if d % 2 == 0 else -sinT[f]) * qs
    ukv = np.empty((128, L), dtype=FP32)
    wkv = np.empty((128, L), dtype=FP32)
    for p in range(64):
        f = p // 2
        ukv[p] = cosT[f]
        wkv[p] = sinT[f] if p % 2 == 0 else -sinT[f]
    ukv[64:] = 1.0
    wkv[64:] = 0.0

    mask = np.triu(np.ones((128, 128), dtype=FP32)).astype(BF16)
    ident = np.eye(128, dtype=FP32).astype(BF16)
    e0 = np.zeros((2, 128), dtype=FP32)
    e0[0, 0:64] = 1.0
    e0[1, 64:128] = 1.0
    wot = pack128(wo.T.reshape(KT, 128, DIM)).astype(BF16)

    in_maps = []
    for r in range(R):
        wq_sh = wq[r * CL:(r + 1) * CL]          # [256, 2048]
        wk_sh = wk[r * HD:(r + 1) * HD]          # [64, 2048]
        wv_sh = wv[r * HD:(r + 1) * HD]
        wcomb = np.concatenate([wq_sh.T, wk_sh.T, wv_sh.T], axis=1)  # [2048, 384]
        wcomb = pack128(wcomb.reshape(KT, 128, 384)).astype(BF16)
        in_maps.append({
            "xt3": xt3, "wcomb": wcomb,
            "uq": uq.astype(BF16), "wq": wq_t.astype(BF16),
            "ukv": ukv.astype(BF16), "wkv": wkv.astype(BF16),
            "mask": mask, "ident": ident, "e0": e0.astype(BF16),
            "wot": wot,
        })
    return in_maps


def run(inputs, trace=False, trace_cores=None):
    nc = _build()
    in_maps = _prep_inputs(**inputs)
    res = run_bass_kernel_spmd(
        nc, in_maps, core_ids=list(range(R)), trace=trace,
        trace_cores=trace_cores,
    )
    shards = [np.asarray(res.results[r]["out"], dtype=FP32) for r in range(R)]
    y = np.empty((BL, DIM), dtype=FP32)
    for r in range(R):
        y[256 * r:256 * (r + 1)] = shards[r][0:256]
        y[L + 256 * r:L + 256 * (r + 1)] = shards[r][256:512]
    return y.reshape(B, L, DIM), res


def kernel(x, freqs_cis, wq, wk, wv, wo):
    y, _ = run(dict(x=x, freqs_cis=freqs_cis, wq=wq, wk=wk, wv=wv, wo=wo))
    return y
